# revision 12
# baseline (speedup 1.0000x reference)
"""DecoderRNN (GRU + embedding + vocab projection) Bass kernel for 8 trn2 cores.

Model (per reference):
  toks = [2, x[0..S-2]]                          (teacher forcing, S=64, B=64)
  e_s  = relu(emb[toks_s])                       (E=512, padding row 0 = 0)
  GRU: r = sig(e@Wir^T + b_ir + h@Whr^T + b_hr)
       z = sig(e@Wiz^T + b_iz + h@Whz^T + b_hz)
       n = tanh(e@Win^T + b_in + r*(h@Whn^T + b_hn))
       h' = (1-z)*n + z*h                        (H=1024)
  logits_s = h_s @ Wout^T + b_out                (V=32000)
  out = logits.transpose(1,0,2)[None]            -> (1, B, S, V) f32

Distribution: the GRU recurrence is inherently serial, so every core runs
it redundantly; the output projection is vocab-split 8 ways (4000 cols per
core) with its matmuls statically interleaved into the recurrence.

v4 structure (per-core):
 - h-part (h@W_hh) matmuls: fp16, batch-stacked A/B lanes (two concurrent
   64-wide PE column groups) -- this is at fp16 peak.
 - input gates gi = relu(emb[toks])@W_ih^T: hoisted OUT of the serial loop
   into a batched full-width GEMM over 128-token tiles, run in fp8e4
   DoubleRow mode (2x MAC rate at M=128). Result is scaled back 1/256 by
   a scalar-engine Identity-activation copy, bias-folded on GpSimd, then
   reshuffled into the batch-stacked per-step layout by SBUF->SBUF DMA.
 - h^T for the next step's matmuls comes from an SBUF->SBUF transpose DMA
   (off the PE; the old PE transpose cost ~56us).
 - output jobs read hT directly as two 64-wide step lanes (A=step 2t,
   B=step 2t+1), removing the hs staging copy.
 - h is carried f16; output written f16 (host widens to f32).

Layouts ("stacked" = batch folded into 128 partitions as two 512-wide
halves; partition p <-> (half=p//64, b=p%64)):
  psum_rz [128,1024]: cols j<512 -> gate r col 512*half+j, j>=512 -> z
  psum_hn [128,512]: col j -> gate n col 512*half+j
  hT [128, 4, 128] f16: [p, c, 64*hh+b] = h[b, 128*(c+4*hh)+p]
  gist [128, 1536] f16 (per step): [64*hh+b, 512*g+j] = gate g col 512*hh+j
"""

import sys

sys.path.insert(0, "/opt/trn_rl_repo")

import numpy as np
import ml_dtypes

import concourse.bass as bass
import concourse.bacc as bacc
import concourse.mybir as mybir
import concourse.tile as tile
from concourse.bass_utils import run_bass_kernel_spmd

FP16 = mybir.dt.float16
FP8 = mybir.dt.float8e4
F32 = mybir.dt.float32
I32 = mybir.dt.int32
DR = mybir.MatmulPerfMode.DoubleRow
IDENT = mybir.ActivationFunctionType.Identity

S, B, H, E, V = 64, 64, 1024, 512, 32000
NCORES = 8
VC = V // NCORES          # 4000 vocab cols per core
SB = S * B                # 4096
KH = H // 128             # 8 h k-chunks
KE = E // 128             # 4 e k-chunks
NN = 8                    # output n-chunks per core
NW = VC // NN             # 500 cols per n-chunk
NT = SB // 128            # 32 output row tiles
WSCALE = 256.0            # fp8 gi-weight scale (undone by ACT copy)
GI_LOOKAHEAD = 2          # gi tile computed 2 tiles (4 steps) ahead
E_LOOKAHEAD = 4           # embedding gathered 4 tiles ahead

_CACHE = {}


def _build(n_steps=S, with_jobs=True):
    key = ("nc", n_steps, with_jobs)
    if key in _CACHE:
        return _CACHE[key]

    nc = bacc.Bacc("TRN2", target_bir_lowering=False, debug=False)

    def din(name, shape, dt):
        return nc.dram_tensor(name, shape, dt, kind="ExternalInput").ap()

    emb_d = din("emb_t", [V, E], FP16)
    idx_d = din("idx", [128, NT], I32)
    w_rzA_d = din("w_rzA", [128, KH, 1024], FP16)
    w_rzB_d = din("w_rzB", [128, KH, 1024], FP16)
    w_hnA_d = din("w_hnA", [128, KH, 512], FP16)
    w_hnB_d = din("w_hnB", [128, KH, 512], FP16)
    wgi_d = din("wgi", [128, KE, 2048], FP8)
    wgn_d = din("wgn", [128, KE, 1024], FP16)
    bias_gi_d = din("bias_gi", [128, 3072], FP16)
    bias_nhh_d = din("bias_nhh", [128, 512], F32)
    hT0_d = din("hT0", [128, 4, 128], FP16)
    h0st_d = din("h0st", [128, 512], FP16)
    w_outT_d = din("w_outT", [128, KH, VC], FP16)
    b_out_d = din("b_out_bc", [128, VC], FP16)
    out_d = nc.dram_tensor("out", [SB, VC], FP16, kind="ExternalOutput").ap()

    n_tiles = (n_steps + 1) // 2

    with tile.TileContext(nc) as tc:
        with tc.tile_pool(name="const", bufs=1) as pc, \
             tc.tile_pool(name="roll", bufs=1) as pr, \
             tc.tile_pool(name="psum", bufs=1, space="PSUM") as pp:

            # ---- constants in SBUF
            w_rzA = pc.tile([128, KH, 1024], FP16, name="w_rzA")
            w_rzB = pc.tile([128, KH, 1024], FP16, name="w_rzB")
            w_hnA = pc.tile([128, KH, 512], FP16, name="w_hnA")
            w_hnB = pc.tile([128, KH, 512], FP16, name="w_hnB")
            wgi = pc.tile([128, KE, 2048], FP8, name="wgi")
            wgn = pc.tile([128, KE, 1024], FP16, name="wgn")
            bias_gi = pc.tile([128, 3072], FP16, name="bias_gi")
            bias_nhh = pc.tile([128, 512], F32, name="bias_nhh")
            w_outT = pc.tile([128, KH, VC], FP16, name="w_outT")
            b_out = pc.tile([128, VC], FP16, name="b_out")
            idx = pc.tile([128, NT], I32, name="idx")

            for t, d in [(w_rzA, w_rzA_d), (w_rzB, w_rzB_d), (w_hnA, w_hnA_d),
                         (w_hnB, w_hnB_d), (wgi, wgi_d), (wgn, wgn_d),
                         (bias_gi, bias_gi_d),
                         (bias_nhh, bias_nhh_d), (w_outT, w_outT_d),
                         (b_out, b_out_d), (idx, idx_d)]:
                nc.sync.dma_start(out=t[:], in_=d[:])

            # ---- embedding gather -> eT8 (fp8 lhsT for the gi GEMM)
            def gather_tile(g):
                er = pr.tile([128, E], FP16, name=f"er{g}", tag="er", bufs=3)
                nc.gpsimd.indirect_dma_start(
                    out=er[:], out_offset=None,
                    in_=emb_d[:],
                    in_offset=bass.IndirectOffsetOnAxis(ap=idx[:, g:g + 1], axis=0),
                )
                eT = pr.tile([128, KE, 128], FP16, name=f"eT{g}", tag="eT",
                             bufs=E_LOOKAHEAD)
                nc.sync.dma_start_transpose(out=eT[:], in_=er[:])
                eT8 = pr.tile([128, KE, 128], FP8, name=f"eT8_{g}", tag="eT8",
                              bufs=E_LOOKAHEAD)
                nc.vector.tensor_copy(out=eT8[:], in_=eT[:])
                return eT, eT8

            eT_w = {g: gather_tile(g) for g in range(min(E_LOOKAHEAD, n_tiles))}

            # ---- batched gi GEMM for one token tile (fp8 DoubleRow, M=128)
            gist_w = {}   # step -> [128, 1536] f16 stacked gi (+biases)

            def gi_tile(g):
                eT, eT8 = eT_w[g]
                gi_sb = pr.tile([128, 3072], FP16, name=f"gisb{g}", tag="gisb",
                                bufs=2)
                for pa in range(2):          # r,z: fp8 DoubleRow, scaled x256
                    ps_gi = pp.tile([128, 1024], F32, name=f"psgi{g}_{pa}",
                                    tag="psgi", bufs=1)
                    for ch in range(2):
                        for p in range(2):
                            nc.tensor.matmul(
                                out=ps_gi[:, 512 * ch:512 * ch + 512],
                                lhsT=eT8[:, 2 * p:2 * p + 2, :],
                                rhs=wgi[:, 2 * p:2 * p + 2,
                                        1024 * pa + 512 * ch:
                                        1024 * pa + 512 * ch + 512],
                                start=(p == 0), stop=(p == 1), perf_mode=DR,
                                skip_group_check=True)
                    nc.scalar.activation(
                        out=gi_sb[:, 1024 * pa:1024 * pa + 1024], in_=ps_gi[:],
                        func=IDENT, scale=1.0 / WSCALE)
                # n gate: fp16 (tanh path is the quantization-sensitive one)
                ps_gn = pp.tile([128, 1024], F32, name=f"psgn{g}", tag="psgi",
                                bufs=1)
                for ch in range(2):
                    for c in range(KE):
                        nc.tensor.matmul(
                            out=ps_gn[:, 512 * ch:512 * ch + 512],
                            lhsT=eT[:, c, :],
                            rhs=wgn[:, c, 512 * ch:512 * ch + 512],
                            start=(c == 0), stop=(c == KE - 1),
                            skip_group_check=True)
                nc.scalar.activation(out=gi_sb[:, 2048:3072], in_=ps_gn[:],
                                     func=IDENT)
                nc.gpsimd.tensor_tensor(out=gi_sb[:], in0=gi_sb[:],
                                        in1=bias_gi[:], op=mybir.AluOpType.add)
                # reshuffle [token, gate] -> per-step stacked [64hh+b, 512g+j]
                for hp in range(2):          # step-half within the tile
                    st = 2 * g + hp
                    if st >= n_steps:
                        break
                    gist = pr.tile([128, 1536], FP16, name=f"gist{st}",
                                   tag="gist", bufs=5)
                    src = gi_sb[64 * hp:64 * hp + 64, :].rearrange(
                        "p (g c) -> p g c", g=3)
                    for hh in range(2):      # hidden half -> dst partitions
                        nc.sync.dma_start(
                            out=gist[64 * hh:64 * hh + 64, :].rearrange(
                                "p (g j) -> p g j", g=3),
                            in_=src[:, :, 512 * hh:512 * hh + 512])
                    gist_w[st] = gist

            for g in range(min(GI_LOOKAHEAD, n_tiles)):
                gi_tile(g)

            hT = pr.tile([128, 4, 128], FP16, name="hT_init", tag="hT", bufs=6)
            h_st = pr.tile([128, 512], FP16, name="hst_init", tag="hst", bufs=2)
            nc.sync.dma_start(out=hT[:], in_=hT0_d[:])
            nc.sync.dma_start(out=h_st[:], in_=h0st_d[:])

            hT_s = {-1: hT}   # step -> hT produced at END of that step... see below
            # hT_s[s] holds h AFTER step s (input to step s+1); hT_s[-1] = h0.

            jobs = [(t, nn) for t in range(n_steps // 2) for nn in range(NN)]
            if not with_jobs:
                jobs = []
            jp = 0

            def emit_job(t, nn):
                ps_o = pp.tile([128, NW], F32, name=f"pso{t}_{nn}", tag="pso",
                               bufs=3)
                hA, hB = hT_s[2 * t], hT_s[2 * t + 1]
                for k in range(KH):
                    lA = hA[:, k % 4, 64 * (k // 4):64 * (k // 4) + 64]
                    lB = hB[:, k % 4, 64 * (k // 4):64 * (k // 4) + 64]
                    st, sp = (k == 0), (k == KH - 1)
                    nc.tensor.matmul(out=ps_o[0:64, :], lhsT=lA,
                                     rhs=w_outT[:, k, nn * NW:(nn + 1) * NW],
                                     start=st, stop=sp, skip_group_check=True)
                    nc.tensor.matmul(out=ps_o[64:128, :], lhsT=lB,
                                     rhs=w_outT[:, k, nn * NW:(nn + 1) * NW],
                                     start=st, stop=sp, skip_group_check=True)
                ob = pr.tile([128, NW], FP16, name=f"ob{t}_{nn}", tag="ob", bufs=4)
                nc.vector.tensor_tensor(
                    out=ob[:], in0=ps_o[:], in1=b_out[:, nn * NW:(nn + 1) * NW],
                    op=mybir.AluOpType.add)
                nc.sync.dma_start(
                    out=out_d[t * 128:(t + 1) * 128, nn * NW:(nn + 1) * NW],
                    in_=ob[:])

            for s in range(n_steps):
                g, half = s // 2, s % 2
                hT_prev = hT_s[s - 1]

                # ---- h-part matmuls fp16 (A/B 64-wide concurrent lanes)
                ps_rz = pp.tile([128, 1024], F32, name=f"psrz{s}", tag="psrz",
                                bufs=1)
                ps_hn = pp.tile([128, 512], F32, name=f"pshn{s}", tag="pshn",
                                bufs=1)
                for k in range(KH):
                    lh = hT_prev[:, k % 4, 64 * (k // 4):64 * (k // 4) + 64]
                    st, sp = (k == 0), (k == KH - 1)
                    nc.tensor.matmul(out=ps_rz[0:64, 0:512], lhsT=lh,
                                     rhs=w_rzA[:, k, 0:512], start=st, stop=sp,
                                     skip_group_check=True)
                    nc.tensor.matmul(out=ps_rz[64:128, 0:512], lhsT=lh,
                                     rhs=w_rzB[:, k, 0:512], start=st, stop=sp,
                                     skip_group_check=True)
                    nc.tensor.matmul(out=ps_rz[0:64, 512:1024], lhsT=lh,
                                     rhs=w_rzA[:, k, 512:1024], start=st, stop=sp,
                                     skip_group_check=True)
                    nc.tensor.matmul(out=ps_rz[64:128, 512:1024], lhsT=lh,
                                     rhs=w_rzB[:, k, 512:1024], start=st, stop=sp,
                                     skip_group_check=True)
                    nc.tensor.matmul(out=ps_hn[0:64, :], lhsT=lh,
                                     rhs=w_hnA[:, k, :], start=st, stop=sp,
                                     skip_group_check=True)
                    nc.tensor.matmul(out=ps_hn[64:128, :], lhsT=lh,
                                     rhs=w_hnB[:, k, :], start=st, stop=sp,
                                     skip_group_check=True)

                # ---- gate chain (gi + biases come from the gist window)
                gist = gist_w[s]
                nc.vector.tensor_tensor(out=ps_rz[:], in0=ps_rz[:],
                                        in1=gist[:, 0:1024],
                                        op=mybir.AluOpType.add)
                rz = pr.tile([128, 1024], FP16, name=f"rz{s}", tag="rz", bufs=2)
                nc.scalar.activation(out=rz[:], in_=ps_rz[:],
                                     func=mybir.ActivationFunctionType.Sigmoid)
                nc.vector.tensor_tensor(out=ps_hn[:], in0=ps_hn[:],
                                        in1=bias_nhh[:], op=mybir.AluOpType.add)
                tn = pr.tile([128, 512], F32, name=f"tn{s}", tag="tn", bufs=2)
                nc.vector.tensor_tensor(out=tn[:], in0=rz[:, 0:512], in1=ps_hn[:],
                                        op=mybir.AluOpType.mult)
                nc.vector.tensor_tensor(out=tn[:], in0=tn[:],
                                        in1=gist[:, 1024:1536],
                                        op=mybir.AluOpType.add)
                n_sb = pr.tile([128, 512], F32, name=f"n{s}", tag="n", bufs=2)
                nc.scalar.activation(out=n_sb[:], in_=tn[:],
                                     func=mybir.ActivationFunctionType.Tanh)
                d_sb = pr.tile([128, 512], F32, name=f"d{s}", tag="d", bufs=2)
                nc.vector.tensor_tensor(out=d_sb[:], in0=h_st[:], in1=n_sb[:],
                                        op=mybir.AluOpType.subtract)
                nc.vector.tensor_tensor(out=d_sb[:], in0=rz[:, 512:1024],
                                        in1=d_sb[:], op=mybir.AluOpType.mult)
                h_st = pr.tile([128, 512], FP16, name=f"hst{s}", tag="hst",
                               bufs=2)
                nc.vector.tensor_tensor(out=h_st[:], in0=n_sb[:], in1=d_sb[:],
                                        op=mybir.AluOpType.add)

                # ---- h -> hT via SBUF->SBUF transpose DMA (off the PE)
                hT_new = pr.tile([128, 4, 128], FP16, name=f"hT{s}", tag="hT",
                                 bufs=6)
                nc.sync.dma_start_transpose(out=hT_new[:], in_=h_st[:])
                hT_s[s] = hT_new

                # ---- output jobs fill the PE while the gate chain runs
                if s >= 2:
                    for _ in range(5):
                        if jp < len(jobs) and 2 * jobs[jp][0] + 2 <= s:
                            emit_job(*jobs[jp])
                            jp += 1

                # ---- lookahead: gather embeddings / compute gi tiles
                if half == 1:
                    if g + E_LOOKAHEAD < n_tiles:
                        eT_w[g + E_LOOKAHEAD] = gather_tile(g + E_LOOKAHEAD)
                    if g + GI_LOOKAHEAD < n_tiles:
                        gi_tile(g + GI_LOOKAHEAD)

            # ---- drain remaining output jobs
            while jp < len(jobs):
                emit_job(*jobs[jp])
                jp += 1

    nc.compile()
    _CACHE[key] = nc
    return nc


def _prep_in_maps(x, hidden, emb, w_ih, w_hh, b_ih, b_hh, w_out, b_out):
    f16, f32 = np.float16, np.float32
    f8 = ml_dtypes.float8_e4m3

    toks = np.concatenate([np.full((1, B), 2, dtype=np.int64),
                           np.asarray(x)[:-1].astype(np.int64)], axis=0)
    t_flat = toks.reshape(SB).astype(np.int32)
    idx = np.ascontiguousarray(t_flat.reshape(NT, 128).T)        # [128, 32]

    emb_t = np.asarray(emb, dtype=f32).copy()
    emb_t[0] = 0.0
    emb_t = np.maximum(emb_t, 0.0).astype(f16)                    # relu folded

    w_hh = np.asarray(w_hh, dtype=f32)
    w_ih = np.asarray(w_ih, dtype=f32)
    Wr, Wz, Wn = w_hh[0:H], w_hh[H:2 * H], w_hh[2 * H:3 * H]

    def kview16(m, kc):  # [rows, K] -> [128, kc, rows] f16 (K on partitions)
        return np.ascontiguousarray(
            m.T.reshape(kc, 128, m.shape[0]).transpose(1, 0, 2)).astype(f16)

    w_rzA = kview16(np.concatenate([Wr[0:512], Wz[0:512]], 0), KH)
    w_rzB = kview16(np.concatenate([Wr[512:1024], Wz[512:1024]], 0), KH)
    w_hnA = kview16(Wn[0:512], KH)
    w_hnB = kview16(Wn[512:1024], KH)

    # gi GEMM weights, gate-major cols: r,z fp8 x256; n fp16 natural
    wgi = np.ascontiguousarray(
        (w_ih[0:2 * H].T * WSCALE).reshape(KE, 128, 2 * H)
        .transpose(1, 0, 2)).astype(f8)
    wgn = np.ascontiguousarray(
        w_ih[2 * H:3 * H].T.reshape(KE, 128, H).transpose(1, 0, 2)).astype(f16)

    b_ih = np.asarray(b_ih, dtype=f32)
    b_hh = np.asarray(b_hh, dtype=f32)
    # bias_gi (broadcast over token rows): r,z get b_ih+b_hh; n gets b_ih only
    bg = np.concatenate([(b_ih + b_hh)[0:2 * H], b_ih[2 * H:3 * H]])
    bias_gi = np.ascontiguousarray(
        np.broadcast_to(bg.astype(f16), (128, 3 * H)))
    bias_nhh = np.empty((128, 512), f32)
    for hp in (0, 1):
        bias_nhh[64 * hp:64 * hp + 64] = \
            b_hh[2 * H:3 * H][512 * hp:512 * hp + 512][None, :]

    h0 = np.asarray(hidden, dtype=f32)[0]                         # [B, H]
    # hT0[p, c, 64*hh+b] = h0[b, 128*(c+4*hh)+p]
    hT0 = np.ascontiguousarray(
        h0.T.reshape(2, 4, 128, B).transpose(2, 1, 0, 3).reshape(128, 4, 128)
    ).astype(f16)
    h0st = np.concatenate([h0[:, 0:512], h0[:, 512:1024]], axis=0).astype(f16)

    w_out = np.asarray(w_out, dtype=f32)
    b_out = np.asarray(b_out, dtype=f32)

    shared = dict(
        emb_t=emb_t, idx=idx,
        w_rzA=w_rzA, w_rzB=w_rzB, w_hnA=w_hnA, w_hnB=w_hnB,
        wgi=wgi, wgn=wgn, bias_gi=bias_gi, bias_nhh=bias_nhh,
        hT0=hT0, h0st=h0st,
    )
    in_maps = []
    for c in range(NCORES):
        sl = slice(c * VC, (c + 1) * VC)
        w_outT = np.ascontiguousarray(
            w_out[sl].T.reshape(KH, 128, VC).transpose(1, 0, 2)).astype(f16)
        b_out_bc = np.ascontiguousarray(
            np.broadcast_to(b_out[sl], (128, VC))).astype(f16)
        in_maps.append(dict(shared, w_outT=w_outT, b_out_bc=b_out_bc))
    return in_maps


def _assemble(results):
    full = np.concatenate(
        [r["out"].astype(np.float32).reshape(S, B, VC) for r in results],
        axis=2)                                                  # (S, B, V)
    return np.ascontiguousarray(full.transpose(1, 0, 2)[None])


def _run(trace=False, tmpdir=None, **inputs):
    nc = _build()
    in_maps = _prep_in_maps(**inputs)
    res = run_bass_kernel_spmd(nc, in_maps, list(range(NCORES)),
                               trace=trace, tmpdir=tmpdir)
    return _assemble(res.results), res


def kernel(**inputs) -> np.ndarray:
    out, _ = _run(**inputs)
    return out


if __name__ == "__main__":
    rng = np.random.default_rng(0)
    ins = dict(
        x=rng.integers(0, V, (S, B)).astype(np.int32),
        hidden=rng.standard_normal((1, B, H)).astype(np.float32),
        emb=rng.standard_normal((V, E)).astype(np.float32),
        w_ih=rng.uniform(-1 / 32, 1 / 32, (3 * H, E)).astype(np.float32),
        w_hh=rng.uniform(-1 / 32, 1 / 32, (3 * H, H)).astype(np.float32),
        b_ih=rng.uniform(-1 / 32, 1 / 32, (3 * H,)).astype(np.float32),
        b_hh=rng.uniform(-1 / 32, 1 / 32, (3 * H,)).astype(np.float32),
        w_out=rng.uniform(-1 / 32, 1 / 32, (V, H)).astype(np.float32),
        b_out=rng.uniform(-1 / 32, 1 / 32, (V,)).astype(np.float32),
    )
    out = kernel(**ins)
    print("out", out.shape, out.dtype, float(np.abs(out).max()))
